# revision 12
# baseline (speedup 1.0000x reference)
"""Trainium2 Bass kernel for single-head cross-attention with additive mask.

Computation (matches the reference):
    q = tgt @ wq + bq
    k = src @ wk (+ bk dropped: softmax cancels a per-row constant exactly)
    v = src @ wv (bv folded into the epilogue: out = attn@v + bv)
    s = (q k^T + mask) / sqrt(DQ)
    out = softmax(s) @ v + bv

Single fused SPMD launch on 8 cores, all fp16 data paths (fp32 PSUM):
  Phase A: core c projects kT and v (+softmax ones column) for its 1/8 of
      the global (B*S) src rows from a host-pre-transposed fp16 src slice,
      then AllGathers the K/V shards through internal DRAM (the collective
      runs on TOPSP/SDMA, overlapping the tgt/mask loads and Q projection).
  Phase B: core c computes attention for tgt rows [c*512,(c+1)*512) of
      every batch, so its fp16 mask slice is read from HBM exactly once.

The attention loop is src-block outer with both batch-pairs inside: the two
pairs' QK matmuls (contraction 64) run concurrently on disjoint PE row
halves via tile_position (0,0)/(64,0).  Mask adds use a stride-0 broadcast
AP (no duplicated mask in SBUF) and are split DVE/GpSimd so neither engine
paces the loop; a single FD=2048 ACT exp per src block covers both pairs.
PV accumulates fp32 in PSUM with the ones-column denominator.  The output
leaves transposed [B, DQ, TS]; the host flips it.
"""
import numpy as np

B, S, D, DQ = 4, 4096, 1024, 64
NCORES = 8
TS = S // NCORES            # 512 tgt rows per core
SR = (B * S) // NCORES      # 2048 global src rows per core (phase A)
SB = S // 128               # 32 src blocks per batch
KL = SR // 128              # 16 src blocks per core (phase A)
KE = DQ * SR                # 131072 fp16 elems of kT per core
VE = 128 * KL * (DQ + 1)    # 133120 fp16 elems of v65 per core
KVE = KE + VE
CORES = list(range(NCORES))
F32 = np.float32
FP16 = np.float16

_CACHE = {}


def _build_fused():
    import concourse.mybir as mybir
    import concourse.tile as tile
    from concourse import bacc

    f32 = mybir.dt.float32
    fp16 = mybir.dt.float16
    AF = mybir.ActivationFunctionType

    nc = bacc.Bacc("TRN2", target_bir_lowering=False, debug=False,
                   num_devices=NCORES)
    srcT = nc.dram_tensor("srcT", [D, SR], fp16, kind="ExternalInput")
    tgtT = nc.dram_tensor("tgtT", [B, D, TS], fp16, kind="ExternalInput")
    # host-transposed mask slice: masknT[s, t] = mask[c*TS + t, s]
    masknT = nc.dram_tensor("masknT", [S, TS], fp16, kind="ExternalInput")
    wq = nc.dram_tensor("wq", [D, DQ], fp16, kind="ExternalInput")
    wk = nc.dram_tensor("wk", [D, DQ], fp16, kind="ExternalInput")
    wv = nc.dram_tensor("wv", [D, DQ], fp16, kind="ExternalInput")
    bq = nc.dram_tensor("bq", [DQ], f32, kind="ExternalInput")
    bv = nc.dram_tensor("bv", [DQ], f32, kind="ExternalInput")
    # transposed output: host flips [B, DQ, TS] -> [B, TS, DQ]
    out = nc.dram_tensor("out", [B, DQ, TS], f32, kind="ExternalOutput")
    # K/V shard exchange through internal DRAM: [kT | v65] per core
    kv_in = nc.dram_tensor("kv_in", [KVE], fp16, kind="Internal")
    kv_out = nc.dram_tensor("kv_out", [NCORES, KVE], fp16, kind="Internal",
                            addr_space="Shared")

    with tile.TileContext(nc) as tc:
        with (
            tc.tile_pool(name="const", bufs=1) as constp,
            tc.tile_pool(name="big", bufs=1) as bigp,
            tc.tile_pool(name="stream", bufs=2) as streamp,
            tc.tile_pool(name="pp", bufs=1, space="PSUM") as pp,
        ):
            # ---- constants (sync ring) ----
            wk_sb = constp.tile([128, 8 * DQ], fp16)
            nc.sync.dma_start(
                out=wk_sb.rearrange("p (j m) -> p j m", m=DQ),
                in_=wk.rearrange("(j p) m -> p j m", p=128))
            wv_sb = constp.tile([128, 8 * DQ], fp16)
            nc.sync.dma_start(
                out=wv_sb.rearrange("p (j m) -> p j m", m=DQ),
                in_=wv.rearrange("(j p) m -> p j m", p=128))
            wq_sb = constp.tile([128, 8 * DQ], fp16)
            nc.sync.dma_start(
                out=wq_sb.rearrange("p (j m) -> p j m", m=DQ),
                in_=wq.rearrange("(j p) m -> p j m", p=128))
            bq_sb = constp.tile([128, 1], f32)
            nc.sync.dma_start(out=bq_sb[0:64, :],
                              in_=bq.rearrange("(p o) -> p o", o=1))
            nc.sync.dma_start(out=bq_sb[64:128, :],
                              in_=bq.rearrange("(p o) -> p o", o=1))
            bv_sb = constp.tile([64, 1], f32)
            nc.sync.dma_start(out=bv_sb[:], in_=bv.rearrange("(p o) -> p o", o=1))

            # ---- mask loads (gpsimd/SWDGE ring, lands during phase A);
            # em = exp(mask/8) is built on ACT during the ramp and the raw
            # mask is never kept: the softmax uses the multiplicative form
            # exp((qk+m)/8) = exp(qk/8) * em.
            em_sb = bigp.tile([128, SB * TS], fp16)
            for g in range(8):
                msc = streamp.tile([128, 4 * TS], fp16, tag="msc", bufs=2,
                                   name=f"msc{g}")
                nc.gpsimd.dma_start(
                    out=msc.rearrange("p (sb t) -> p sb t", t=TS),
                    in_=masknT[g * 512:(g + 1) * 512, :]
                    .rearrange("(sb p) t -> p sb t", p=128))
                nc.scalar.activation(
                    em_sb[:, g * 4 * TS:(g + 1) * 4 * TS], msc[:],
                    AF.Exp, scale=0.125)

            # ---- phase A: K/V projection of this core's src shard ----
            v_sb = bigp.tile([128, KL * (DQ + 1)], fp16)
            nc.vector.memset(
                v_sb.rearrange("p (k c) -> p k c", c=DQ + 1)[:, :, DQ:DQ + 1],
                1.0)
            kT_psA = pp.tile([128, 1024], f32, tag="qk0")
            kT_psB = pp.tile([128, 1024], f32, tag="qk1")
            v_ps = [pp.tile([128, 4 * DQ], f32, tag=f"pv{q // 2}{q % 2}",
                            name=f"v_ps{q}") for q in range(4)]
            for j in range(8):
                st = streamp.tile([128, SR], fp16, tag="xs", bufs=4,
                                  name=f"st{j}")
                # split src across both HWDGE rings to halve phase-A latency
                eng = nc.sync if j % 2 == 0 else nc.scalar
                eng.dma_start(out=st[:], in_=srcT[j * 128:(j + 1) * 128, :])
                for g in (0, 2, 1, 3):  # alternate col-groups for PE overlap
                    if g < 2:
                        ps, col, tp, po = kT_psA, g * 512, (0, 0), 0
                    else:
                        ps, col, tp, po = kT_psB, (g - 2) * 512, (0, 64), 64
                    nc.tensor.matmul(
                        ps[po:po + 64, col:col + 512],
                        lhsT=wk_sb[:, j * DQ:(j + 1) * DQ],
                        rhs=st[:, g * 512:(g + 1) * 512],
                        start=(j == 0), stop=(j == 7), tile_position=tp)
                for k in range(16):
                    nc.tensor.matmul(
                        v_ps[k // 4][:, (k % 4) * DQ:(k % 4 + 1) * DQ],
                        lhsT=st[:, k * 128:(k + 1) * 128],
                        rhs=wv_sb[:, j * DQ:(j + 1) * DQ],
                        start=(j == 0 and k % 4 == 0),
                        stop=(j == 7 and k % 4 == 3))
            kT_sb = bigp.tile([128, 1024], fp16)
            nc.scalar.copy(kT_sb[0:64, :], kT_psA[0:64, :])
            nc.scalar.copy(kT_sb[64:128, :], kT_psB[64:128, :])
            v_view = v_sb.rearrange("p (k c) -> p k c", c=DQ + 1)
            for q in range(4):
                nc.vector.tensor_copy(
                    v_view[:, 4 * q:4 * (q + 1), 0:DQ],
                    v_ps[q].rearrange("p (k c) -> p k c", c=DQ))
            # shard -> kv_in (scalar ring keeps sync free for tgt loads)
            kvk = kv_in[0:KE].rearrange("(p t) -> p t", p=DQ)
            nc.scalar.dma_start(out=kvk[:, 0:1024], in_=kT_sb[0:64, :])
            nc.scalar.dma_start(out=kvk[:, 1024:2048], in_=kT_sb[64:128, :])
            nc.scalar.dma_start(
                out=kv_in[KE:KVE].rearrange("(p t) -> p t", p=128),
                in_=v_sb[:])

            # ---- AllGather K/V (TOPSP/SDMA; overlaps everything below) ----
            nc.gpsimd.collective_compute(
                "AllGather", mybir.AluOpType.bypass,
                replica_groups=[CORES],
                ins=[kv_in[:]], outs=[kv_out[:, :]])

            # ---- tgt loads (sync ring, right behind src) ----
            tgs = []
            for b in (0, 2, 1, 3):
                tg = streamp.tile([128, 8 * TS], fp16, tag="xs", bufs=4,
                                  name=f"tg{b}")
                nc.sync.dma_start(
                    out=tg.rearrange("p (j t) -> p j t", t=TS),
                    in_=tgtT[b].rearrange("(j p) t -> p j t", p=128))
                tgs.append((b, tg))

            # ---- gathered K/V -> SBUF (scalar ring, after the collective) --
            # kT2: partitions 0-63 = d of batches 0-1; 64-127 = batches 2-3
            kT2 = bigp.tile([128, 2 * S], fp16)
            for c in (0, 2, 4, 6, 1, 3, 5, 7):  # sg 0-15 sources first
                po = 64 * (c // 4)
                co = ((c // 2) % 2) * S + (c % 2) * SR
                nc.scalar.dma_start(
                    out=kT2[po:po + 64, co:co + SR],
                    in_=kv_out[c, 0:KE].rearrange("(p t) -> p t", p=DQ))
            v2 = bigp.tile([128, B * SB * (DQ + 1)], fp16)
            VQ = KL * (DQ + 1)
            for c in CORES:
                nc.scalar.dma_start(
                    out=v2[:, c * VQ:(c + 1) * VQ],
                    in_=kv_out[c, KE:KVE].rearrange("(p t) -> p t", p=128))

            # ---- qT projection: batches (0,2) then (1,3) col-concurrent ----
            qT_sb = bigp.tile([128, 2 * TS], fp16)
            q_ps = {}
            for b, tg in tgs:
                q_ps[b] = pp.tile([128, TS], f32, tag=f"qk{b % 2}",
                                  name=f"q_ps{b}")
            for wave in ((0, 2), (1, 3)):
                for j in range(8):
                    for b in wave:
                        pb = (b // 2) * 64
                        nc.tensor.matmul(
                            q_ps[b][pb:pb + 64, :],
                            lhsT=wq_sb[:, j * DQ:(j + 1) * DQ],
                            rhs=dict(tgs)[b][:, j * TS:(j + 1) * TS],
                            start=(j == 0), stop=(j == 7),
                            tile_position=(0, pb))
            for b in range(B):
                pb, colb = (b // 2) * 64, (b % 2) * TS
                nc.scalar.activation(
                    qT_sb[pb:pb + 64, colb:colb + TS], q_ps[b][pb:pb + 64, :],
                    AF.Identity, bias=bq_sb[pb:pb + 64, :])

            # ---- attention main loop ----
            pv_ps = [[pp.tile([65, TS], f32, tag=f"pv{pair}{h}",
                              name=f"pv_ps{pair}_{h}") for h in range(2)]
                     for pair in range(2)]
            qk_tag = ["qk0", "qk1"]
            mul_eng = [nc.vector, nc.gpsimd]
            for sg in range(SB):
                qk = [pp.tile([128, 2 * TS], f32, tag=qk_tag[pair],
                              name=f"qkt{pair}_{sg}") for pair in range(2)]
                for half in range(2):
                    for pair in range(2):
                        pb = pair * 64
                        nc.tensor.matmul(
                            qk[pair][:, half * TS:(half + 1) * TS],
                            lhsT=kT2[pb:pb + 64, half * S + sg * 128:
                                     half * S + sg * 128 + 128],
                            rhs=qT_sb[pb:pb + 64, half * TS:(half + 1) * TS],
                            start=True, stop=True, tile_position=(pb, 0))
                ems = em_sb[:, sg * TS:(sg + 1) * TS]
                pt = streamp.tile([128, 4 * TS], fp16, tag="P", bufs=3,
                                  name=f"pt{sg}")
                for pair in range(2):
                    ptr = streamp.tile([128, 2 * TS], fp16, tag=f"R{pair}",
                                       bufs=3, name=f"ptr{pair}_{sg}")
                    nc.scalar.activation(ptr[:], qk[pair][:], AF.Exp,
                                         scale=0.125)
                    for h in range(2):  # per-half: all step-1 fp16 = DVE 2x
                        # gpsimd is ~2x slower per element: give it 1 of 4
                        eng = nc.gpsimd if (pair, h) == (1, 1) else nc.vector
                        eng.tensor_mul(
                            pt[:, (pair * 2 + h) * TS:(pair * 2 + h + 1) * TS],
                            ptr[:, h * TS:(h + 1) * TS],
                            ems)
                for pair in range(2):
                    for half in range(2):
                        kg = (pair * 2 + half) * SB + sg
                        nc.tensor.matmul(
                            pv_ps[pair][half][:],
                            lhsT=v2[:, kg * (DQ + 1):(kg + 1) * (DQ + 1)],
                            rhs=pt[:, (pair * 2 + half) * TS:
                                   (pair * 2 + half + 1) * TS],
                            start=(sg == 0), stop=(sg == SB - 1))

            # ---- epilogue: out^T = pv[0:64]/sums + bv (batched, PE-free) ---
            sums = streamp.tile([65, 4 * TS], f32, tag="sums", bufs=1)
            for pair in range(2):
                for half in range(2):
                    b = pair * 2 + half
                    nc.scalar.copy(sums[64:65, b * TS:(b + 1) * TS],
                                   pv_ps[pair][half][64:65, :])
            sums0 = streamp.tile([1, 4 * TS], f32, tag="sums0", bufs=1)
            nc.sync.dma_start(out=sums0[:], in_=sums[64:65, :])
            recip = streamp.tile([1, 4 * TS], f32, tag="recip", bufs=1)
            nc.vector.reciprocal_approx_fast(recip[:], sums0[:])
            rb = streamp.tile([64, 4 * TS], f32, tag="sums", bufs=1)
            nc.gpsimd.partition_broadcast(rb[:], recip[:])
            for pair in range(2):
                for half in range(2):
                    b = pair * 2 + half
                    ot = streamp.tile([64, TS], f32, tag="ot", bufs=2)
                    nc.vector.tensor_mul(ot[:], pv_ps[pair][half][0:64, :],
                                         rb[:, b * TS:(b + 1) * TS])
                    of = streamp.tile([64, TS], f32, tag="of", bufs=2)
                    nc.vector.tensor_scalar_add(of[:], ot[:], bv_sb[:])
                    eng = nc.sync if half == 0 else nc.scalar
                    eng.dma_start(out=out[b], in_=of[:])
    nc.compile()
    return nc


def _get_fused():
    if "fused" not in _CACHE:
        _CACHE["fused"] = _build_fused()
    return _CACHE["fused"]


def make_in_maps(src, tgt, mask, wq, bq, wk, wv, bv):
    src_flat = np.asarray(src, dtype=F32).reshape(B * S, D)
    tgt = np.asarray(tgt, dtype=F32)
    mask = np.asarray(mask, dtype=F32)
    wq16 = np.ascontiguousarray(np.asarray(wq, dtype=F32).astype(FP16))
    wk16 = np.ascontiguousarray(np.asarray(wk, dtype=F32).astype(FP16))
    wv16 = np.ascontiguousarray(np.asarray(wv, dtype=F32).astype(FP16))
    bq = np.ascontiguousarray(bq, dtype=F32)
    bv = np.ascontiguousarray(bv, dtype=F32)
    return [{
        "srcT": np.ascontiguousarray(
            src_flat[c * SR:(c + 1) * SR, :].T.astype(FP16)),
        "tgtT": np.ascontiguousarray(
            tgt[:, c * TS:(c + 1) * TS, :].transpose(0, 2, 1).astype(FP16)),
        "masknT": np.ascontiguousarray(
            mask[c * TS:(c + 1) * TS, :].T.astype(FP16)),
        "wq": wq16, "wk": wk16, "wv": wv16, "bq": bq, "bv": bv,
    } for c in CORES]


def kernel(src, tgt, mask, wq, bq, wk, bk, wv, bv):
    from concourse.bass_utils import run_bass_kernel_spmd

    res = run_bass_kernel_spmd(
        _get_fused(), make_in_maps(src, tgt, mask, wq, bq, wk, wv, bv),
        core_ids=CORES)
    out = np.empty((B, S, DQ), dtype=F32)
    for c in CORES:
        out[:, c * TS:(c + 1) * TS, :] = \
            np.asarray(res.results[c]["out"]).transpose(0, 2, 1)
    return out


# revision 13
# speedup vs baseline: 1.5507x; 1.5507x over previous
"""Trainium2 Bass kernel for single-head cross-attention with additive mask.

Computation (matches the reference):
    q = tgt @ wq + bq
    k = src @ wk (+ bk dropped: softmax cancels a per-row constant exactly)
    v = src @ wv (bv folded into the epilogue: out = attn@v + bv)
    s = (q k^T + mask) / sqrt(DQ)
    out = softmax(s) @ v + bv

Two SPMD launches on 8 cores, all fp16 data paths (fp32 PSUM accumulate):
  L1: each core projects kT and v for 1/8 of the global (B*S) src rows from a
      host-pre-transposed fp16 src slice.  Outputs are written in the exact
      layout L2 consumes (kt [64, 2048]; v65 [128, 16*65] with the softmax
      ones-column baked in), so the host glue is pure array passing.
  L2: tgt sharded 8 ways; core c handles tgt rows [c*512,(c+1)*512) of every
      batch so its mask slice is read from HBM exactly once (fp16, host-cast).

L2 attention loop is src-block inner, batch-pair concurrent: the two pairs'
QK matmuls (contraction 64) run on disjoint PE row-halves via tile_position
(0,0)/(64,0) so they execute concurrently.  The mask add runs on DVE with a
stride-0 broadcast AP (no duplicated mask in SBUF), writing an fp16 es tile;
one FD=2048 ACT exp per src block covers both pairs.  PV accumulates fp32 in
PSUM with the ones-column denominator trick.  The output leaves transposed
[B, DQ, TS]; the host flips it.
"""
import numpy as np

B, S, D, DQ = 4, 4096, 1024, 64
NCORES = 8
TS = S // NCORES            # 512 tgt rows per core
SR = (B * S) // NCORES      # 2048 global src rows per core (L1)
SB = S // 128               # 32 src blocks per batch
KL = SR // 128              # 16 src blocks per core (L1)
CORES = list(range(NCORES))
F32 = np.float32
FP16 = np.float16

_CACHE = {}


def _build_l1():
    import concourse.mybir as mybir
    import concourse.tile as tile
    from concourse import bacc

    f32 = mybir.dt.float32
    fp16 = mybir.dt.float16

    nc = bacc.Bacc("TRN2", target_bir_lowering=False, debug=False,
                   num_devices=NCORES)
    srcT = nc.dram_tensor("srcT", [D, SR], fp16, kind="ExternalInput")
    wk = nc.dram_tensor("wk", [D, DQ], fp16, kind="ExternalInput")
    wv = nc.dram_tensor("wv", [D, DQ], fp16, kind="ExternalInput")
    # kt[:, 0:1024] = kT of src rows 0-1023 (this core), [:, 1024:] rows 1024+
    kt = nc.dram_tensor("kt", [DQ, SR], fp16, kind="ExternalOutput")
    # v65[p, k*65 + c] = v[k*128 + p, c] for c<64; ones at c=64
    v65 = nc.dram_tensor("v65", [128, KL * (DQ + 1)], fp16,
                         kind="ExternalOutput")

    with tile.TileContext(nc) as tc:
        with (
            tc.tile_pool(name="const", bufs=1) as constp,
            tc.tile_pool(name="big", bufs=1) as bigp,
            tc.tile_pool(name="stream", bufs=2) as streamp,
            tc.tile_pool(name="pp", bufs=1, space="PSUM") as pp,
        ):
            wk_sb = constp.tile([128, 8 * DQ], fp16)
            nc.sync.dma_start(
                out=wk_sb.rearrange("p (j m) -> p j m", m=DQ),
                in_=wk.rearrange("(j p) m -> p j m", p=128))
            wv_sb = constp.tile([128, 8 * DQ], fp16)
            nc.sync.dma_start(
                out=wv_sb.rearrange("p (j m) -> p j m", m=DQ),
                in_=wv.rearrange("(j p) m -> p j m", p=128))

            v_sb = bigp.tile([128, KL * (DQ + 1)], fp16)
            nc.vector.memset(
                v_sb.rearrange("p (k c) -> p k c", c=DQ + 1)[:, :, DQ:DQ + 1],
                1.0)

            kT_psA = pp.tile([128, 1024], f32, tag="qk0")
            kT_psB = pp.tile([128, 1024], f32, tag="qk1")
            v_ps = [pp.tile([128, 4 * DQ], f32, tag=f"pv{q}", name=f"v_ps{q}")
                    for q in range(4)]
            for j in range(8):
                st = streamp.tile([128, SR], fp16, tag="xs", bufs=3)
                nc.sync.dma_start(out=st[:], in_=srcT[j * 128:(j + 1) * 128, :])
                for g in (0, 2, 1, 3):  # alternate col-groups for PE overlap
                    if g < 2:
                        ps, col, tp, po = kT_psA, g * 512, (0, 0), 0
                    else:
                        ps, col, tp, po = kT_psB, (g - 2) * 512, (0, 64), 64
                    nc.tensor.matmul(
                        ps[po:po + 64, col:col + 512],
                        lhsT=wk_sb[:, j * DQ:(j + 1) * DQ],
                        rhs=st[:, g * 512:(g + 1) * 512],
                        start=(j == 0), stop=(j == 7), tile_position=tp)
                for k in range(16):
                    nc.tensor.matmul(
                        v_ps[k // 4][:, (k % 4) * DQ:(k % 4 + 1) * DQ],
                        lhsT=st[:, k * 128:(k + 1) * 128],
                        rhs=wv_sb[:, j * DQ:(j + 1) * DQ],
                        start=(j == 0 and k % 4 == 0),
                        stop=(j == 7 and k % 4 == 3))
            kT_sb = bigp.tile([128, 1024], fp16)
            nc.scalar.copy(kT_sb[0:64, :], kT_psA[0:64, :])
            nc.scalar.copy(kT_sb[64:128, :], kT_psB[64:128, :])
            nc.sync.dma_start(out=kt[:, 0:1024], in_=kT_sb[0:64, :])
            nc.sync.dma_start(out=kt[:, 1024:2048], in_=kT_sb[64:128, :])
            v_view = v_sb.rearrange("p (k c) -> p k c", c=DQ + 1)
            for q in range(4):
                nc.vector.tensor_copy(
                    v_view[:, 4 * q:4 * (q + 1), 0:DQ],
                    v_ps[q].rearrange("p (k c) -> p k c", c=DQ))
            nc.sync.dma_start(out=v65[:, :], in_=v_sb[:])
    nc.compile()
    return nc


def _build_l2():
    import concourse.mybir as mybir
    import concourse.tile as tile
    from concourse import bacc

    f32 = mybir.dt.float32
    fp16 = mybir.dt.float16
    AF = mybir.ActivationFunctionType

    nc = bacc.Bacc("TRN2", target_bir_lowering=False, debug=False,
                   num_devices=NCORES)
    # per-core L1 outputs, fed straight through (no host reshaping)
    kts = [nc.dram_tensor(f"kt_{c}", [DQ, SR], fp16, kind="ExternalInput")
           for c in CORES]
    v65s = [nc.dram_tensor(f"v65_{c}", [128, KL * (DQ + 1)], fp16,
                           kind="ExternalInput") for c in CORES]
    tgtT = nc.dram_tensor("tgtT", [B, D, TS], fp16, kind="ExternalInput")
    # host-transposed mask slice: masknT[s, t] = mask[c*TS + t, s]
    masknT = nc.dram_tensor("masknT", [S, TS], fp16, kind="ExternalInput")
    wq = nc.dram_tensor("wq", [D, DQ], fp16, kind="ExternalInput")
    bq = nc.dram_tensor("bq", [DQ], f32, kind="ExternalInput")
    bv = nc.dram_tensor("bv", [DQ], f32, kind="ExternalInput")
    # transposed output: host flips [B, DQ, TS] -> [B, TS, DQ]
    out = nc.dram_tensor("out", [B, DQ, TS], f32, kind="ExternalOutput")

    with tile.TileContext(nc) as tc:
        with (
            tc.tile_pool(name="const", bufs=1) as constp,
            tc.tile_pool(name="big", bufs=1) as bigp,
            tc.tile_pool(name="stream", bufs=2) as streamp,
            tc.tile_pool(name="pp", bufs=1, space="PSUM") as pp,
        ):
            wq_sb = constp.tile([128, 8 * DQ], fp16)
            nc.sync.dma_start(
                out=wq_sb.rearrange("p (j m) -> p j m", m=DQ),
                in_=wq.rearrange("(j p) m -> p j m", p=128))
            bq_sb = constp.tile([128, 1], f32)
            nc.sync.dma_start(out=bq_sb[0:64, :],
                              in_=bq.rearrange("(p o) -> p o", o=1))
            nc.sync.dma_start(out=bq_sb[64:128, :],
                              in_=bq.rearrange("(p o) -> p o", o=1))
            bv_sb = constp.tile([64, 1], f32)
            nc.sync.dma_start(out=bv_sb[:], in_=bv.rearrange("(p o) -> p o", o=1))

            # kT2 layout: partitions 0-63 = d, s of batches 0-1; 64-127 = 2-3
            kT2 = bigp.tile([128, 2 * S], fp16)
            for c in CORES:
                po = 64 * (c // 4)
                co = ((c // 2) % 2) * S + (c % 2) * SR
                nc.sync.dma_start(out=kT2[po:po + 64, co:co + SR],
                                  in_=kts[c][:, :])
            # v65 on the scalar HWDGE ring (parallel FIFO to sync's)
            v2 = bigp.tile([128, B * SB * (DQ + 1)], fp16)
            VQ = KL * (DQ + 1)
            for c in CORES:
                nc.scalar.dma_start(out=v2[:, c * VQ:(c + 1) * VQ],
                                    in_=v65s[c][:, :])
            # tgt loads + mask first chunk early; rest of mask after
            tgs = []
            for b in (0, 2, 1, 3):
                tg = streamp.tile([128, 8 * TS], fp16, tag="xs", bufs=4,
                                  name=f"tg{b}")
                nc.sync.dma_start(
                    out=tg.rearrange("p (j t) -> p j t", t=TS),
                    in_=tgtT[b].rearrange("(j p) t -> p j t", p=128))
                tgs.append((b, tg))
            # mask, fp16 from host: [128 s-partitions, (sb, t)]
            maskTd = bigp.tile([128, SB * TS], fp16)
            mview = maskTd.rearrange("p (sb t) -> p sb t", t=TS)
            for g in range(4):
                nc.sync.dma_start(
                    out=mview[:, g * 8:(g + 1) * 8, :],
                    in_=masknT[g * 1024:(g + 1) * 1024, :]
                    .rearrange("(sb p) t -> p sb t", p=128))

            # qT projection: batches (0,2) then (1,3) in col-concurrent pairs
            qT_sb = bigp.tile([128, 2 * TS], fp16)
            q_ps = {}
            for b, tg in tgs:
                pb = (b // 2) * 64
                q_ps[b] = pp.tile([128, TS], f32, tag=f"qk{b % 2}",
                                  name=f"q_ps{b}")
            for wave in ((0, 2), (1, 3)):
                for j in range(8):
                    for b in wave:
                        pb = (b // 2) * 64
                        tg = dict(tgs)[b]
                        nc.tensor.matmul(
                            q_ps[b][pb:pb + 64, :],
                            lhsT=wq_sb[:, j * DQ:(j + 1) * DQ],
                            rhs=tg[:, j * TS:(j + 1) * TS],
                            start=(j == 0), stop=(j == 7),
                            tile_position=(0, pb))
            for b in range(B):
                pb, colb = (b // 2) * 64, (b % 2) * TS
                nc.scalar.activation(
                    qT_sb[pb:pb + 64, colb:colb + TS], q_ps[b][pb:pb + 64, :],
                    AF.Identity, bias=bq_sb[pb:pb + 64, :])

            # attention main loop: src-block outer, both batch-pairs inside.
            # QK for pair0/pair1 run concurrently on PE row-halves.
            pv_ps = [[pp.tile([65, TS], f32, tag=f"pv{pair}{h}",
                              name=f"pv_ps{pair}_{h}") for h in range(2)]
                     for pair in range(2)]
            qk_tag = ["qk0", "qk1"]
            for sg in range(SB):
                qk = [pp.tile([128, 2 * TS], f32, tag=qk_tag[pair],
                              name=f"qkt{pair}_{sg}") for pair in range(2)]
                for half in range(2):
                    for pair in range(2):
                        pb = pair * 64
                        nc.tensor.matmul(
                            qk[pair][:, half * TS:(half + 1) * TS],
                            lhsT=kT2[pb:pb + 64, half * S + sg * 128:
                                     half * S + sg * 128 + 128],
                            rhs=qT_sb[pb:pb + 64, half * TS:(half + 1) * TS],
                            start=True, stop=True, tile_position=(pb, 0))
                es = streamp.tile([128, 4 * TS], fp16, tag="E", bufs=3,
                                  name=f"es{sg}")
                mb = (maskTd[:, sg * TS:(sg + 1) * TS]
                      .rearrange("p (h t) -> p h t", h=1)
                      .broadcast_to([128, 2, TS]))
                for pair in range(2):
                    nc.vector.tensor_add(
                        es[:, pair * 2 * TS:(pair + 1) * 2 * TS]
                        .rearrange("p (h t) -> p h t", h=2),
                        qk[pair].rearrange("p (h t) -> p h t", h=2),
                        mb)
                pt = streamp.tile([128, 4 * TS], fp16, tag="P", bufs=3,
                                  name=f"pt{sg}")
                nc.scalar.activation(pt[:], es[:], AF.Exp, scale=0.125)
                for pair in range(2):
                    for half in range(2):
                        b = pair * 2 + half
                        kg = b * SB + sg
                        nc.tensor.matmul(
                            pv_ps[pair][half][:],
                            lhsT=v2[:, kg * (DQ + 1):(kg + 1) * (DQ + 1)],
                            rhs=pt[:, (pair * 2 + half) * TS:
                                   (pair * 2 + half + 1) * TS],
                            start=(sg == 0), stop=(sg == SB - 1))

            # epilogue: out^T = pv[0:64]/sums + bv, all PE-free
            for pair in range(2):
                for half in range(2):
                    b = pair * 2 + half
                    sums = streamp.tile([65, TS], f32, tag="sums")
                    nc.scalar.copy(sums[64:65, :], pv_ps[pair][half][64:65, :])
                    sums0 = streamp.tile([1, TS], f32, tag="sums0")
                    nc.sync.dma_start(out=sums0[:], in_=sums[64:65, :])
                    recip = streamp.tile([1, TS], f32, tag="recip")
                    rscr = streamp.tile([1, TS], f32, tag="rscr")
                    nc.vector.reciprocal_approx_accurate(recip[:], sums0[:],
                                                         rscr[:])
                    rb = streamp.tile([64, TS], f32, tag="rb")
                    nc.gpsimd.partition_broadcast(rb[:], recip[:])
                    ot = streamp.tile([64, TS], f32, tag="ot")
                    nc.vector.tensor_mul(ot[:], pv_ps[pair][half][0:64, :],
                                         rb[:])
                    of = streamp.tile([64, TS], f32, tag="of")
                    nc.scalar.activation(of[:], ot[:], AF.Identity,
                                         bias=bv_sb[:])
                    nc.sync.dma_start(out=out[b], in_=of[:])
    nc.compile()
    return nc


def _get_l1():
    if "l1" not in _CACHE:
        _CACHE["l1"] = _build_l1()
    return _CACHE["l1"]


def _get_l2():
    if "l2" not in _CACHE:
        _CACHE["l2"] = _build_l2()
    return _CACHE["l2"]


def make_in_maps_l1(src, wk, wv):
    src_flat = np.asarray(src, dtype=F32).reshape(B * S, D)
    wk = np.ascontiguousarray(wk, dtype=FP16)
    wv = np.ascontiguousarray(wv, dtype=FP16)
    return [{
        "srcT": np.ascontiguousarray(
            src_flat[c * SR:(c + 1) * SR, :].T.astype(FP16)),
        "wk": wk, "wv": wv,
    } for c in CORES]


def make_in_maps_l2(res1, tgt, mask, wq, bq, bv):
    tgt = np.asarray(tgt, dtype=F32)
    mask = np.asarray(mask, dtype=F32)
    wq = np.ascontiguousarray(np.asarray(wq, dtype=F32).astype(FP16))
    bq = np.ascontiguousarray(bq, dtype=F32)
    bv = np.ascontiguousarray(bv, dtype=F32)
    shared = {}
    for c in CORES:
        shared[f"kt_{c}"] = np.asarray(res1[c]["kt"])
        shared[f"v65_{c}"] = np.asarray(res1[c]["v65"])
    return [{
        **shared,
        "tgtT": np.ascontiguousarray(
            tgt[:, c * TS:(c + 1) * TS, :].transpose(0, 2, 1).astype(FP16)),
        "masknT": np.ascontiguousarray(
            mask[c * TS:(c + 1) * TS, :].T.astype(FP16)),
        "wq": wq, "bq": bq, "bv": bv,
    } for c in CORES]


def kernel(src, tgt, mask, wq, bq, wk, bk, wv, bv):
    from concourse.bass_utils import run_bass_kernel_spmd

    res1 = run_bass_kernel_spmd(_get_l1(), make_in_maps_l1(src, wk, wv),
                                core_ids=CORES)
    res2 = run_bass_kernel_spmd(
        _get_l2(), make_in_maps_l2(res1.results, tgt, mask, wq, bq, bv),
        core_ids=CORES)
    out = np.empty((B, S, DQ), dtype=F32)
    for c in CORES:
        out[:, c * TS:(c + 1) * TS, :] = \
            np.asarray(res2.results[c]["out"]).transpose(0, 2, 1)
    return out
